# revision 6
# baseline (speedup 1.0000x reference)
"""CROMA dense transformer on 8 Trainium2 NeuronCores.

Data-parallel: core c computes batch item c end-to-end (no collectives).
Activations live feature-major ("T layout": [D on partitions, tokens on free])
so every linear layer is psum = W_chunk.T @ xT with weights stationary and no
activation transposes anywhere. Weights are cast to bf16 on host; LN gamma/beta
and foldable biases are folded into the weights on host. Heads are padded
48->64 for clean partition-slicing of Q/K; V stays token-natural with a fused
ones-column so the softmax denominator falls out of the AV matmul for free.
"""

import os
import numpy as np
import ml_dtypes

import concourse.bass as bass
import concourse.mybir as mybir
from concourse import bacc
from concourse.tile import TileContext
from concourse.masks import make_identity
from concourse.bass_utils import run_bass_kernel_spmd

# ---------------------------------------------------------------- constants
DIM = 768
HEADS = 16
DH = 48
DHP = 64  # padded head dim
PATCH = 8
RES = 120
NP = 225          # tokens
NPAD = 256        # padded keys
S1_DEPTH = 6
S2_DEPTH = 12
CROSS_DEPTH = 6
B = 8
P = 128
KT = DIM // P     # 6
FFN = 4 * DIM     # 3072
FKT = FFN // P    # 24
SCALE = DH ** -0.5
BIAS_NEG = -30000.0
EPS = 1e-5

dt = mybir.dt
AF = mybir.ActivationFunctionType
ALU = mybir.AluOpType
AX = mybir.AxisListType
bf16 = ml_dtypes.bfloat16

N_CORES = 8


def _depths():
    env = os.environ.get("CROMA_DEPTHS")
    if env:
        a, b, c = (int(v) for v in env.split(","))
        return a, b, c
    return S1_DEPTH, S2_DEPTH, CROSS_DEPTH


# ---------------------------------------------------------------- host prep
def _np(x):
    return np.asarray(x, dtype=np.float32)


def _pad_heads_cols(w):
    out = np.zeros((w.shape[0], HEADS * DHP), np.float32)
    for h in range(HEADS):
        out[:, h * DHP : h * DHP + DH] = w[:, h * DH : (h + 1) * DH]
    return out


def _pad_heads_vec(b):
    out = np.zeros((HEADS * DHP,), np.float32)
    for h in range(HEADS):
        out[h * DHP : h * DHP + DH] = b[h * DH : (h + 1) * DH]
    return out


def _pad_heads_rows(w):
    out = np.zeros((HEADS * DHP, w.shape[1]), np.float32)
    for h in range(HEADS):
        out[h * DHP : h * DHP + DH, :] = w[h * DH : (h + 1) * DH, :]
    return out


def prep_attn(p):
    g, bta = _np(p['ln']['g']), _np(p['ln']['b'])
    w = _np(p['qkv']['w'])
    wq, wk, wv = w[:, :DIM] * SCALE, w[:, DIM:2 * DIM], w[:, 2 * DIM:]
    wo, bo = _np(p['out']['w']), _np(p['out']['b'])
    wqk = np.concatenate([_pad_heads_cols(g[:, None] * wq),
                          _pad_heads_cols(g[:, None] * wk)], axis=1)
    bqk = np.concatenate([_pad_heads_vec(bta @ wq), _pad_heads_vec(bta @ wk)])
    return {
        'wqk': wqk.astype(bf16),
        'bqk': bqk,
        'wv': (g[:, None] * wv).astype(bf16),
        'wo': _pad_heads_rows(wo).astype(bf16),
        'bo': bo + (bta @ wv) @ wo,
    }


def prep_xattn(p):
    g, bta = _np(p['ln']['g']), _np(p['ln']['b'])
    wq = _np(p['q']['w']) * SCALE
    wk = _np(p['k']['w'])
    wv = _np(p['v']['w'])
    wo, bo = _np(p['out']['w']), _np(p['out']['b'])
    return {
        'wq': _pad_heads_cols(g[:, None] * wq).astype(bf16),
        'bq': _pad_heads_vec(bta @ wq),
        'wk': _pad_heads_cols(g[:, None] * wk).astype(bf16),
        'bk': _pad_heads_vec(bta @ wk),
        'wv': (g[:, None] * wv).astype(bf16),
        'wo': _pad_heads_rows(wo).astype(bf16),
        'bo': bo + (bta @ wv) @ wo,
    }


def prep_ffn(p):
    g, bta = _np(p['ln']['g']), _np(p['ln']['b'])
    w1, b1 = _np(p['fc1']['w']), _np(p['fc1']['b'])
    w2, b2 = _np(p['fc2']['w']), _np(p['fc2']['b'])
    return {
        'w1': (g[:, None] * w1).astype(bf16),
        'b1': bta @ w1 + b1,
        'w2': w2.astype(bf16),
        'b2': b2,
    }


def patchify(img):
    """[C, 120, 120] -> [C*64, 225] fp32 (x0 transposed)."""
    C = img.shape[0]
    hh = RES // PATCH
    x = img.reshape(C, hh, PATCH, hh, PATCH).transpose(1, 3, 0, 2, 4)
    x = x.reshape(hh * hh, C * PATCH * PATCH)
    return np.ascontiguousarray(x.T.astype(np.float32))


def prep_alibi(attn_bias):
    b = _np(attn_bias)[0]
    out = np.full((P, 2 * HEADS, NP), BIAS_NEG, np.float32)
    for h in range(HEADS):
        bt = b[h].T
        out[:, 2 * h + 0, :] = bt[0:128, :]
        out[0:97, 2 * h + 1, :] = bt[128:225, :]
    return out.astype(bf16)


class BiasBlob:
    def __init__(self):
        self.cols = []
        self.offsets = {}

    def add(self, name, vec):
        vec = _np(vec)
        kt = vec.shape[0] // P
        self.offsets[name] = (len(self.cols), kt)
        arr = vec.reshape(kt, P).T
        for j in range(kt):
            self.cols.append(arr[:, j])

    def plan(self, name, n):
        kt = n // P
        self.offsets[name] = (len(self.cols), kt)
        for _ in range(kt):
            self.cols.append(None)

    def array(self):
        return np.stack(self.cols, axis=1).astype(np.float32)


def _blob_walk(blob, d1, d2, dc, add_fns=None):
    """Shared layout walk; add_fns = dict of callables to emit data."""
    A = add_fns

    def put(name, n, data_fn=None):
        if A is None:
            blob.plan(name, n)
        else:
            blob.add(name, data_fn())

    return put


def build_host_inputs(sar_imgs, opt_imgs, attn_bias, params, depths):
    d1, d2, dc = depths
    blob = BiasBlob()
    shared = {}

    def add_w(name, arr):
        shared[name] = np.ascontiguousarray(arr)

    def do_vit(tag, vp, depth):
        add_w(f'{tag}_inw', _np(vp['in']['w']).astype(bf16))
        blob.add(f'{tag}_inb', _np(vp['in']['b']))
        for i, L in enumerate(vp['layers'][:depth]):
            a = prep_attn(L['attn'])
            f = prep_ffn(L['ffn'])
            add_w(f'{tag}{i}a_wqk', a['wqk']); blob.add(f'{tag}{i}a_bqk', a['bqk'])
            add_w(f'{tag}{i}a_wv', a['wv'])
            add_w(f'{tag}{i}a_wo', a['wo']); blob.add(f'{tag}{i}a_bo', a['bo'])
            add_w(f'{tag}{i}f_w1', f['w1']); blob.add(f'{tag}{i}f_b1', f['b1'])
            add_w(f'{tag}{i}f_w2', f['w2']); blob.add(f'{tag}{i}f_b2', f['b2'])
        blob.add(f'{tag}_lng', _np(vp['ln']['g']))
        blob.add(f'{tag}_lnb', _np(vp['ln']['b']))

    do_vit('s1', params['s1'], d1)
    do_vit('s2', params['s2'], d2)

    cp = params['cross']
    for i, L in enumerate(cp['layers'][:dc]):
        a = prep_attn(L['sa'])
        x = prep_xattn(L['ca'])
        f = prep_ffn(L['ffn'])
        add_w(f'c{i}s_wqk', a['wqk']); blob.add(f'c{i}s_bqk', a['bqk'])
        add_w(f'c{i}s_wv', a['wv'])
        add_w(f'c{i}s_wo', a['wo']); blob.add(f'c{i}s_bo', a['bo'])
        add_w(f'c{i}x_wq', x['wq']); blob.add(f'c{i}x_bq', x['bq'])
        add_w(f'c{i}x_wk', x['wk']); blob.add(f'c{i}x_bk', x['bk'])
        add_w(f'c{i}x_wv', x['wv'])
        add_w(f'c{i}x_wo', x['wo']); blob.add(f'c{i}x_bo', x['bo'])
        add_w(f'c{i}f_w1', f['w1']); blob.add(f'c{i}f_b1', f['b1'])
        add_w(f'c{i}f_w2', f['w2']); blob.add(f'c{i}f_b2', f['b2'])
    blob.add('c_lng', _np(cp['ln']['g']))
    blob.add('c_lnb', _np(cp['ln']['b']))

    for tag, gp in (('g1', params['s1_gap']), ('g2', params['s2_gap'])):
        g = prep_ffn(gp)
        add_w(f'{tag}_w1', g['w1']); blob.add(f'{tag}_b1', g['b1'])
        add_w(f'{tag}_w2', g['w2']); blob.add(f'{tag}_b2', g['b2'])

    shared['biases'] = blob.array()
    shared['alibi'] = prep_alibi(attn_bias)

    sar = _np(sar_imgs)
    opt = _np(opt_imgs)
    per_core = [{'x0sar': patchify(sar[c]), 'x0opt': patchify(opt[c])}
                for c in range(N_CORES)]
    return shared, per_core, blob.offsets


def plan_bias_offsets(depths):
    d1, d2, dc = depths
    blob = BiasBlob()

    def do_vit(tag, depth):
        blob.plan(f'{tag}_inb', DIM)
        for i in range(depth):
            blob.plan(f'{tag}{i}a_bqk', 2 * HEADS * DHP)
            blob.plan(f'{tag}{i}a_bo', DIM)
            blob.plan(f'{tag}{i}f_b1', FFN)
            blob.plan(f'{tag}{i}f_b2', DIM)
        blob.plan(f'{tag}_lng', DIM)
        blob.plan(f'{tag}_lnb', DIM)

    do_vit('s1', d1)
    do_vit('s2', d2)
    for i in range(dc):
        blob.plan(f'c{i}s_bqk', 2 * HEADS * DHP)
        blob.plan(f'c{i}s_bo', DIM)
        blob.plan(f'c{i}x_bq', HEADS * DHP)
        blob.plan(f'c{i}x_bk', HEADS * DHP)
        blob.plan(f'c{i}x_bo', DIM)
        blob.plan(f'c{i}f_b1', FFN)
        blob.plan(f'c{i}f_b2', DIM)
    blob.plan('c_lng', DIM)
    blob.plan('c_lnb', DIM)
    for tag in ('g1', 'g2'):
        blob.plan(f'{tag}_b1', FFN)
        blob.plan(f'{tag}_b2', DIM)
    return blob.offsets, len(blob.cols)


# ---------------------------------------------------------------- device build
def build_program(depths):
    d1, d2, dc = depths
    bias_off, bias_cols = plan_bias_offsets(depths)
    nc = bacc.Bacc("TRN2", target_bir_lowering=False, debug=False,
                   num_devices=N_CORES)

    din = {}

    def dram_in(name, shape, dtype):
        din[name] = nc.dram_tensor(name, list(shape), dtype, kind="ExternalInput")

    dram_in('x0sar', (P, NP), dt.float32)
    dram_in('x0opt', (DIM, NP), dt.float32)
    dram_in('biases', (P, bias_cols), dt.float32)
    dram_in('alibi', (P, 2 * HEADS, NP), dt.bfloat16)

    def dram_w(name, shape):
        dram_in(name, shape, dt.bfloat16)

    def vit_weights(tag, depth):
        dram_w(f'{tag}_inw', (P if tag == 's1' else DIM, DIM))
        for i in range(depth):
            dram_w(f'{tag}{i}a_wqk', (DIM, 2 * HEADS * DHP))
            dram_w(f'{tag}{i}a_wv', (DIM, DIM))
            dram_w(f'{tag}{i}a_wo', (HEADS * DHP, DIM))
            dram_w(f'{tag}{i}f_w1', (DIM, FFN))
            dram_w(f'{tag}{i}f_w2', (FFN, DIM))

    vit_weights('s1', d1)
    vit_weights('s2', d2)
    for i in range(dc):
        dram_w(f'c{i}s_wqk', (DIM, 2 * HEADS * DHP))
        dram_w(f'c{i}s_wv', (DIM, DIM))
        dram_w(f'c{i}s_wo', (HEADS * DHP, DIM))
        dram_w(f'c{i}x_wq', (DIM, HEADS * DHP))
        dram_w(f'c{i}x_wk', (DIM, HEADS * DHP))
        dram_w(f'c{i}x_wv', (DIM, DIM))
        dram_w(f'c{i}x_wo', (HEADS * DHP, DIM))
        dram_w(f'c{i}f_w1', (DIM, FFN))
        dram_w(f'c{i}f_w2', (FFN, DIM))
    for tag in ('g1', 'g2'):
        dram_w(f'{tag}_w1', (DIM, FFN))
        dram_w(f'{tag}_w2', (FFN, DIM))

    douts = {}
    for name, shape in (('sar_enc', (NP, DIM)), ('opt_enc', (NP, DIM)),
                        ('joint', (NP, DIM)), ('sar_gap', (DIM,)),
                        ('opt_gap', (DIM,)), ('joint_gap', (DIM,))):
        douts[name] = nc.dram_tensor(name, list(shape), dt.float32,
                                     kind="ExternalOutput")

    from contextlib import ExitStack
    import itertools
    _ctr = itertools.count()

    with TileContext(nc) as tc, ExitStack() as es:
        cst = es.enter_context(tc.tile_pool(name="cst", bufs=1))
        persist = es.enter_context(tc.tile_pool(name="persist", bufs=1))
        act = es.enter_context(tc.tile_pool(name="act", bufs=2))
        small = es.enter_context(tc.tile_pool(name="small", bufs=3))
        big = es.enter_context(tc.tile_pool(name="big", bufs=1))
        wp768 = es.enter_context(tc.tile_pool(name="wp768", bufs=16))
        wp1024 = es.enter_context(tc.tile_pool(name="wp1024", bufs=8))
        ps_mm = es.enter_context(tc.tile_pool(name="ps_mm", bufs=3, space="PSUM"))
        ps_sc = es.enter_context(tc.tile_pool(name="ps_sc", bufs=2, space="PSUM"))
        ps_av = es.enter_context(tc.tile_pool(name="ps_av", bufs=2, space="PSUM"))
        ps_bc = es.enter_context(tc.tile_pool(name="ps_bc", bufs=1, space="PSUM"))
        ps_pool = {"mm": ps_mm, "sc": ps_sc, "av": ps_av}

        # ---- constants
        bias_sb = cst.tile([P, bias_cols], dt.float32)
        nc.sync.dma_start(bias_sb[:], din['biases'][:])
        alibi_sb = cst.tile([P, 2 * HEADS, NP], dt.bfloat16)
        nc.sync.dma_start(alibi_sb[:], din['alibi'][:])
        ones_col32 = cst.tile([P, 1], dt.float32)
        nc.vector.memset(ones_col32[:], 1.0)
        ones_col16 = cst.tile([P, 1], dt.bfloat16)
        nc.vector.memset(ones_col16[:], 1.0)
        ones_row32 = cst.tile([1, P], dt.float32)
        nc.vector.memset(ones_row32[:], 1.0)
        ident = cst.tile([P, P], dt.float32)
        make_identity(nc, ident[:])
        eps_sb = cst.tile([1, 1], dt.float32)
        nc.vector.memset(eps_sb[:], EPS)

        VS = DHP + 1  # 65: V cols 0..47, zeros 48..63, ones col at 64
        v_sb = persist.tile([P, 2, HEADS * VS], dt.bfloat16)
        nc.vector.memset(v_sb[:], 0.0)
        nc.vector.memset(
            v_sb[:].rearrange("p t (h c) -> p t h c", c=VS)[:, :, :, DHP], 1.0)
        outT_sb = persist.tile([P, HEADS // 2, NP], dt.bfloat16)
        nc.vector.memset(outT_sb[:], 0.0)

        def bias_ap(name, j):
            off, kt = bias_off[name]
            assert j < kt, (name, j, kt)
            return bias_sb[:, off + j, None]

        # ---------------------------------------------------------- helpers
        def w_group(name, kt_in, c0, c1):
            """DMA chunks [P, c1-c0] for all kt of weight columns [c0:c1)."""
            ap = din[name].rearrange("(ko p) m -> p ko m", p=P)
            width = c1 - c0
            tag = "w768" if width <= DIM else "w1024"
            pool = wp768 if width <= DIM else wp1024
            chunks = []
            for k in range(kt_in):
                c = pool.tile([P, DIM if width <= DIM else 1024],
                              dt.bfloat16, tag=tag,
                              name=f"wchunk_{next(_ctr)}")[:, :width]
                nc.sync.dma_start(c[:], ap[:, k, c0:c1])
                chunks.append(c)
            return chunks

        def linear(xn, name, kt_in, m_out, consumer, group_cols=None):
            """psum_m = sum_k W[:, m-block].T @ xn[:, k, :]; consumer(m, psum).
            Streams weight column-groups of group_cols (default min(m_out,1024))."""
            gc = group_cols or min(m_out, 1024)
            for c0 in range(0, m_out, gc):
                chunks = w_group(name, kt_in, c0, min(c0 + gc, m_out))
                for mi in range((min(c0 + gc, m_out) - c0) // P):
                    m = c0 // P + mi
                    pt = ps_mm.tile([P, 512], dt.float32, tag="mm",
                                    name=f"ptmm_{next(_ctr)}")[:, :NP]
                    for k in range(kt_in):
                        nc.tensor.matmul(pt[:], chunks[k][:, mi * P:(mi + 1) * P],
                                         xn[:, k, :], start=(k == 0),
                                         stop=(k == kt_in - 1))
                    consumer(m, pt)

        def linear_ktg(xn, name, kt_in, m_out, consumer, kgroup=KT,
                       nfree=NP):
            """kt-grouped linear for tall weights (fc2): m_out//P open psums,
            weights streamed kgroup k-chunks at a time."""
            nm = m_out // P
            psums = []
            for m in range(nm):
                pool = (ps_mm, ps_sc, ps_av)[m % 3]
                psums.append(pool.tile([P, 512], dt.float32,
                                       tag=("mm", "sc", "av")[m % 3],
                                       name=f"ptk_{next(_ctr)}")[:, :nfree])
            ap = din[name].rearrange("(ko p) m -> p ko m", p=P)
            for g0 in range(0, kt_in, kgroup):
                for k in range(g0, min(g0 + kgroup, kt_in)):
                    c = wp768.tile([P, DIM], dt.bfloat16, tag="w768")
                    nc.sync.dma_start(c[:], ap[:, k, :])
                    for m in range(nm):
                        nc.tensor.matmul(psums[m][:], c[:, m * P:(m + 1) * P],
                                         xn[:, k, :], start=(k == 0),
                                         stop=(k == kt_in - 1))
            for m in range(nm):
                consumer(m, psums[m])

        def ln_raw(xT, out_dtype=dt.bfloat16, gamma=None, beta=None,
                   out_pool=None, out_tag="xn"):
            pool = out_pool or act
            xsq = act.tile([P, KT, NP], dt.bfloat16, tag="xsq")
            nc.scalar.square(xsq[:], xT[:])
            pstat = ps_bc.tile([1, 512], dt.float32, tag="bc")
            for k in range(KT):
                nc.tensor.matmul(pstat[0:1, 0:NP], ones_col32[:], xT[:, k, :],
                                 start=(k == 0), stop=(k == KT - 1))
            for k in range(KT):
                nc.tensor.matmul(pstat[0:1, NP:2 * NP], ones_col16[:],
                                 xsq[:, k, :], start=(k == 0),
                                 stop=(k == KT - 1))
            st = small.tile([1, 4 * NP], dt.float32, tag="stat")
            mu = st[0:1, 0:NP]
            r = st[0:1, NP:2 * NP]
            e2 = st[0:1, 2 * NP:3 * NP]
            m2 = st[0:1, 3 * NP:4 * NP]
            nc.vector.tensor_scalar_mul(mu, pstat[0:1, 0:NP], 1.0 / DIM)
            nc.vector.tensor_scalar_mul(e2, pstat[0:1, NP:2 * NP], 1.0 / DIM)
            nc.vector.tensor_mul(m2, mu, mu)
            nc.vector.tensor_sub(e2, e2, m2)
            nc.scalar.activation(m2, e2, AF.Sqrt, bias=eps_sb[0:1, 0:1],
                                 scale=1.0)
            nc.vector.reciprocal(r, m2)
            pb = ps_bc.tile([P, 512], dt.float32, tag="bc")
            nc.tensor.matmul(pb[:, 0:2 * NP], ones_row32[0:1, :],
                             st[0:1, 0:2 * NP], start=True, stop=True)
            xn = pool.tile([P, KT, NP], out_dtype, tag=out_tag)
            for k in range(KT):
                nc.vector.tensor_sub(xn[:, k, :], xT[:, k, :], pb[:, 0:NP])
                nc.vector.tensor_mul(xn[:, k, :], xn[:, k, :], pb[:, NP:2 * NP])
            if gamma is not None:
                for k in range(KT):
                    nc.vector.tensor_scalar(xn[:, k, :], xn[:, k, :],
                                            bias_ap(gamma, k), bias_ap(beta, k),
                                            ALU.mult, ALU.add)
            return xn

        def attention(xT, xn, pfx, kv_xn=None, cross=False):
            kv_src = kv_xn if kv_xn is not None else xn
            qkT = big.tile([P, 16, NPAD], dt.bfloat16, tag="qkT")
            nc.vector.memset(qkT[:, :, NP:], 0.0)

            if not cross:
                def qk_consume(m, pt):
                    nc.scalar.activation(qkT[:, m, 0:NP], pt[:], AF.Identity,
                                         bias=bias_ap(f'{pfx}_bqk', m), scale=1.0)
                linear(xn, f'{pfx}_wqk', KT, 2 * HEADS * DHP, qk_consume)
                wo_name, bo_name, wv_name = f'{pfx}_wo', f'{pfx}_bo', f'{pfx}_wv'
            else:
                def q_consume(m, pt):
                    nc.scalar.activation(qkT[:, m, 0:NP], pt[:], AF.Identity,
                                         bias=bias_ap(f'{pfx}_bq', m), scale=1.0)
                linear(xn, f'{pfx}_wq', KT, HEADS * DHP, q_consume)

                def k_consume(m, pt):
                    nc.scalar.activation(qkT[:, 8 + m, 0:NP], pt[:], AF.Identity,
                                         bias=bias_ap(f'{pfx}_bk', m), scale=1.0)
                linear(kv_src, f'{pfx}_wk', KT, HEADS * DHP, k_consume)
                wo_name, bo_name, wv_name = f'{pfx}_wo', f'{pfx}_bo', f'{pfx}_wv'

            # V natural [keys, 768]
            wv_chunks = w_group(wv_name, KT, 0, DIM)
            for mt, rows in ((0, P), (1, NP - P)):
                for nt in range(2):
                    pv = ps_mm.tile([P, 512], dt.float32, tag="mm",
                                    name=f"ptv_{next(_ctr)}")[:, :384]
                    for k in range(KT):
                        nc.tensor.matmul(
                            pv[:rows, :],
                            kv_src[:, k, mt * P: mt * P + rows],
                            wv_chunks[k][:, nt * 384:(nt + 1) * 384],
                            start=(k == 0), stop=(k == KT - 1))
                    for j in range(8):
                        h = nt * 8 + j
                        nc.vector.tensor_copy(
                            v_sb[:rows, mt, h * VS: h * VS + DH],
                            pv[:rows, j * DH:(j + 1) * DH])

            for h in range(HEADS):
                rb = (h % 2) * DHP
                qm, km = h // 2, 8 + h // 2
                po = ps_av.tile([P, NP], dt.float32, tag="av")
                expts = []
                for kt2 in range(2):
                    pscr = ps_sc.tile([P, NP], dt.float32, tag="sc")
                    nc.tensor.matmul(pscr[:],
                                     qkT[rb:rb + DHP, km, kt2 * P:(kt2 + 1) * P],
                                     qkT[rb:rb + DHP, qm, 0:NP],
                                     start=True, stop=True)
                    sT = small.tile([P, NP], dt.bfloat16, tag="sT")
                    nc.vector.tensor_add(sT[:], pscr[:],
                                         alibi_sb[:, 2 * h + kt2, :])
                    eT = small.tile([P, NP], dt.bfloat16, tag="eT")
                    nc.scalar.activation(eT[:], sT[:], AF.Exp)
                    expts.append(eT)
                for kt2 in range(2):
                    nc.tensor.matmul(
                        po[0:VS, :],
                        v_sb[:, kt2, h * VS:(h + 1) * VS],
                        expts[kt2][:], start=(kt2 == 0), stop=(kt2 == 1))
                rec = small.tile([1, NP], dt.float32, tag="rec")
                nc.vector.reciprocal(rec[:], po[DHP:DHP + 1, :])
                pbc = ps_bc.tile([P, 512], dt.float32, tag="bc")
                nc.tensor.matmul(pbc[0:DH, 0:NP], ones_row32[0:1, 0:DH],
                                 rec[0:1, :], start=True, stop=True)
                recb = small.tile([DH, NP], dt.float32, tag="recb")
                nc.scalar.copy(recb[:], pbc[0:DH, 0:NP])
                nc.vector.tensor_mul(outT_sb[rb:rb + DH, h // 2, :],
                                     po[0:DH, :], recb[:])

            xT_new = act.tile([P, KT, NP], dt.float32, tag="xT")

            def out_consume(m, pt):
                nc.vector.scalar_tensor_tensor(xT_new[:, m, :], pt[:],
                                               bias_ap(bo_name, m),
                                               xT[:, m, :], ALU.add, ALU.add)
            linear(outT_sb, wo_name, HEADS * DHP // P, DIM, out_consume,
                   group_cols=DIM)
            return xT_new

        def ffn(xT, pfx):
            xn = ln_raw(xT)
            hT = big.tile([P, FKT, NP], dt.bfloat16, tag="hT")

            def fc1_consume(m, pt):
                nc.scalar.activation(hT[:, m, :], pt[:], AF.Gelu,
                                     bias=bias_ap(f'{pfx}_b1', m), scale=1.0)
            linear(xn, f'{pfx}_w1', KT, FFN, fc1_consume, group_cols=DIM)

            xT_new = act.tile([P, KT, NP], dt.float32, tag="xT")

            def fc2_consume(m, pt):
                nc.vector.scalar_tensor_tensor(xT_new[:, m, :], pt[:],
                                               bias_ap(f'{pfx}_b2', m),
                                               xT[:, m, :], ALU.add, ALU.add)
            linear_ktg(hT, f'{pfx}_w2', FKT, DIM, fc2_consume)
            return xT_new

        def emit_enc_outputs(encT, out_name):
            for m in range(KT):
                for t2, rows in ((0, P), (1, NP - P)):
                    ptr = ps_bc.tile([P, 512], dt.float32, tag="bc")
                    nc.tensor.transpose(ptr[:rows, 0:P],
                                        encT[:, m, t2 * P: t2 * P + rows],
                                        ident[:])
                    nat = small.tile([P, P], dt.float32, tag="nat")
                    nc.vector.tensor_copy(nat[:rows, :], ptr[:rows, 0:P])
                    nc.sync.dma_start(
                        douts[out_name][t2 * P: t2 * P + rows,
                                        m * P:(m + 1) * P],
                        nat[:rows, :])

        def gap_head(encT, tag, out_name):
            mt = small.tile([P, KT, 1], dt.float32, tag="gapm")
            nc.vector.reduce_sum(out=mt[:, :, 0], in_=encT[:], axis=AX.X)
            nc.vector.tensor_scalar_mul(mt[:], mt[:], 1.0 / NP)
            msq = small.tile([P, KT, 1], dt.bfloat16, tag="gapsq")
            nc.scalar.square(msq[:], mt[:])
            pstat = ps_bc.tile([1, 512], dt.float32, tag="bc")
            for k in range(KT):
                nc.tensor.matmul(pstat[0:1, 0:1], ones_col32[:], mt[:, k, :],
                                 start=(k == 0), stop=(k == KT - 1))
            for k in range(KT):
                nc.tensor.matmul(pstat[0:1, 1:2], ones_col16[:], msq[:, k, :],
                                 start=(k == 0), stop=(k == KT - 1))
            st = small.tile([1, 4], dt.float32, tag="gapst")
            mu, r, tmp = st[0:1, 0:1], st[0:1, 1:2], st[0:1, 2:3]
            nc.vector.tensor_scalar_mul(mu, pstat[0:1, 0:1], 1.0 / DIM)
            nc.vector.tensor_scalar_mul(tmp, pstat[0:1, 1:2], 1.0 / DIM)
            m2 = st[0:1, 3:4]
            nc.vector.tensor_mul(m2, mu, mu)
            nc.vector.tensor_sub(tmp, tmp, m2)
            nc.scalar.activation(m2, tmp, AF.Sqrt, bias=eps_sb[0:1, 0:1],
                                 scale=1.0)
            nc.vector.reciprocal(r, m2)
            pb = ps_bc.tile([P, 512], dt.float32, tag="bc")
            nc.tensor.matmul(pb[:, 0:2], ones_row32[0:1, :], st[0:1, 0:2],
                             start=True, stop=True)
            xn = small.tile([P, KT, 1], dt.bfloat16, tag="gapxn")
            for k in range(KT):
                nc.vector.tensor_sub(xn[:, k, :], mt[:, k, :], pb[:, 0:1])
                nc.vector.tensor_mul(xn[:, k, :], xn[:, k, :], pb[:, 1:2])
            hT = small.tile([P, FKT, 1], dt.bfloat16, tag="gaph")

            def g1_consume(m, pt):
                nc.scalar.activation(hT[:, m, 0:1], pt[:, 0:1], AF.Gelu,
                                     bias=bias_ap(f'{tag}_b1', m), scale=1.0)

            def lin1(m_lo, m_hi, chunks):
                for m in range(m_lo, m_hi):
                    pt = ps_mm.tile([P, 512], dt.float32, tag="mm",
                                    name=f"ptg_{next(_ctr)}")[:, 0:1]
                    for k in range(KT):
                        nc.tensor.matmul(pt[:], chunks[k][:, (m - m_lo) * P:
                                                          (m - m_lo + 1) * P],
                                         xn[:, k, :], start=(k == 0),
                                         stop=(k == KT - 1))
                    g1_consume(m, pt)

            for g in range(4):
                chunks = w_group(f'{tag}_w1', KT, g * DIM, (g + 1) * DIM)
                lin1(g * KT, (g + 1) * KT, chunks)

            gout = small.tile([P, KT, 1], dt.float32, tag="gapo")

            def g2_consume(m, pt):
                nc.scalar.activation(gout[:, m, 0:1], pt[:, 0:1], AF.Identity,
                                     bias=bias_ap(f'{tag}_b2', m), scale=1.0)
            linear_ktg(hT, f'{tag}_w2', FKT, DIM, g2_consume, nfree=1)
            nc.sync.dma_start(douts[out_name].rearrange("(ko p) -> p ko", p=P),
                              gout[:, :, 0])

        # ---------------------------------------------------------- encoders
        def run_vit(tag, depth, x0_name, in_kt):
            x0 = persist.tile([P, KT, NP], dt.float32, tag="x0",
                              name=f"x0_{tag}")[:, :in_kt, :]
            nc.sync.dma_start(
                x0[:], din[x0_name].rearrange("(ko p) n -> p ko n", p=P))
            x0b = persist.tile([P, KT, NP], dt.bfloat16, tag="x0b",
                               name=f"x0b_{tag}")[:, :in_kt, :]
            nc.vector.tensor_copy(x0b[:], x0[:])
            xT = act.tile([P, KT, NP], dt.float32, tag="xT")

            def in_consume(m, pt):
                nc.scalar.activation(xT[:, m, :], pt[:], AF.Identity,
                                     bias=bias_ap(f'{tag}_inb', m), scale=1.0)
            linear(x0b, f'{tag}_inw', in_kt, DIM, in_consume, group_cols=DIM)

            for i in range(depth):
                xn = ln_raw(xT)
                xT = attention(xT, xn, f'{tag}{i}a')
                xT = ffn(xT, f'{tag}{i}f')
            encT = ln_raw(xT, out_dtype=dt.float32, gamma=f'{tag}_lng',
                          beta=f'{tag}_lnb', out_pool=persist,
                          out_tag=f'enc_{tag}')
            return encT

        sar_encT = run_vit('s1', d1, 'x0sar', 1)
        emit_enc_outputs(sar_encT, 'sar_enc')
        gap_head(sar_encT, 'g1', 'sar_gap')

        opt_encT = run_vit('s2', d2, 'x0opt', 6)
        emit_enc_outputs(opt_encT, 'opt_enc')
        gap_head(opt_encT, 'g2', 'opt_gap')

        cn_raw = ln_raw(opt_encT, out_pool=persist, out_tag="cn_raw")

        xT = sar_encT
        for i in range(dc):
            xn = ln_raw(xT)
            xT = attention(xT, xn, f'c{i}s')
            xn = ln_raw(xT)
            xT = attention(xT, xn, f'c{i}x', kv_xn=cn_raw, cross=True)
            xT = ffn(xT, f'c{i}f')
        jointT = ln_raw(xT, out_dtype=dt.float32, gamma='c_lng', beta='c_lnb',
                        out_pool=persist, out_tag='enc_joint')
        emit_enc_outputs(jointT, 'joint')

        jm = small.tile([P, KT, 1], dt.float32, tag="gapm")
        nc.vector.reduce_sum(out=jm[:, :, 0], in_=jointT[:], axis=AX.X)
        nc.vector.tensor_scalar_mul(jm[:], jm[:], 1.0 / NP)
        nc.sync.dma_start(douts['joint_gap'].rearrange("(ko p) -> p ko", p=P),
                          jm[:, :, 0])

    nc.finalize()
    return nc


# ---------------------------------------------------------------- entrypoint
_CACHED = {}


def kernel(sar_imgs, opt_imgs, attn_bias, params):
    depths = _depths()
    shared, per_core, _ = build_host_inputs(sar_imgs, opt_imgs, attn_bias,
                                            params, depths)
    if depths not in _CACHED:
        _CACHED[depths] = build_program(depths)
    nc = _CACHED[depths]
    in_maps = [{**shared, **pc} for pc in per_core]
    res = run_bass_kernel_spmd(nc, in_maps, list(range(N_CORES)))
    outs = []
    for name in ('sar_enc', 'sar_gap', 'opt_enc', 'opt_gap', 'joint',
                 'joint_gap'):
        outs.append(np.stack([res.results[c][name] for c in range(N_CORES)]))
    return tuple(outs)


# revision 22
# speedup vs baseline: 153.0535x; 153.0535x over previous
"""CROMA dense transformer on 8 Trainium2 NeuronCores.

Data-parallel: core c computes batch item c end-to-end (no collectives).
Activations live feature-major ("T layout": [D on partitions, tokens on free])
so every linear layer is psum = W_chunk.T @ xT with weights stationary and no
activation transposes anywhere. Weights are cast to bf16 on host; LN gamma/beta
and foldable biases are folded into the weights on host. Heads are padded
48->64 for clean partition-slicing of Q/K; V stays token-natural with a fused
ones-column so the softmax denominator falls out of the AV matmul for free.
"""

import os
import numpy as np
import ml_dtypes

import concourse.bass as bass
import concourse.mybir as mybir
from concourse import bacc
from concourse.tile import TileContext
from concourse.masks import make_identity
from concourse.bass_utils import run_bass_kernel_spmd

# ---------------------------------------------------------------- constants
DIM = 768
HEADS = 16
DH = 48
DHP = 64  # padded head dim
PATCH = 8
RES = 120
NP = 225          # tokens
NPAD = 256        # padded keys
S1_DEPTH = 6
S2_DEPTH = 12
CROSS_DEPTH = 6
B = 8
P = 128
KT = DIM // P     # 6
FFN = 4 * DIM     # 3072
FKT = FFN // P    # 24
SCALE = DH ** -0.5
BIAS_NEG = -30000.0
EPS = 1e-5

dt = mybir.dt
AF = mybir.ActivationFunctionType
ALU = mybir.AluOpType
AX = mybir.AxisListType
bf16 = ml_dtypes.bfloat16

N_CORES = 8

MM_PHASES = []
_CUR_PHASE = ["?"]


def set_phase(p):
    _CUR_PHASE[0] = p


def _depths():
    env = os.environ.get("CROMA_DEPTHS")
    if env:
        a, b, c = (int(v) for v in env.split(","))
        return a, b, c
    return S1_DEPTH, S2_DEPTH, CROSS_DEPTH


# ---------------------------------------------------------------- host prep
def _np(x):
    return np.asarray(x, dtype=np.float32)


def _pad_heads_cols(w):
    out = np.zeros((w.shape[0], HEADS * DHP), np.float32)
    for h in range(HEADS):
        out[:, h * DHP : h * DHP + DH] = w[:, h * DH : (h + 1) * DH]
    return out


def _pad_heads_vec(b):
    out = np.zeros((HEADS * DHP,), np.float32)
    for h in range(HEADS):
        out[h * DHP : h * DHP + DH] = b[h * DH : (h + 1) * DH]
    return out


def _pad_heads_rows(w):
    out = np.zeros((HEADS * DHP, w.shape[1]), np.float32)
    for h in range(HEADS):
        out[h * DHP : h * DHP + DH, :] = w[h * DH : (h + 1) * DH, :]
    return out


def prep_attn(p):
    g, bta = _np(p['ln']['g']), _np(p['ln']['b'])
    w = _np(p['qkv']['w'])
    wq, wk, wv = w[:, :DIM] * SCALE, w[:, DIM:2 * DIM], w[:, 2 * DIM:]
    wo, bo = _np(p['out']['w']), _np(p['out']['b'])
    wqk = np.concatenate([_pad_heads_cols(g[:, None] * wq),
                          _pad_heads_cols(g[:, None] * wk)], axis=1)
    bqk = np.concatenate([_pad_heads_vec(bta @ wq), _pad_heads_vec(bta @ wk)])
    return {
        'wqk': wqk.astype(bf16),
        'bqk': bqk,
        'wv': (g[:, None] * wv).astype(bf16),
        'wo': _pad_heads_rows(wo).astype(bf16),
        'bo': bo + (bta @ wv) @ wo,
    }


def prep_xattn(p):
    g, bta = _np(p['ln']['g']), _np(p['ln']['b'])
    wq = _np(p['q']['w']) * SCALE
    wk = _np(p['k']['w'])
    wv = _np(p['v']['w'])
    wo, bo = _np(p['out']['w']), _np(p['out']['b'])
    return {
        'wq': _pad_heads_cols(g[:, None] * wq).astype(bf16),
        'bq': _pad_heads_vec(bta @ wq),
        'wk': _pad_heads_cols(g[:, None] * wk).astype(bf16),
        'bk': _pad_heads_vec(bta @ wk),
        'wv': (g[:, None] * wv).astype(bf16),
        'wo': _pad_heads_rows(wo).astype(bf16),
        'bo': bo + (bta @ wv) @ wo,
    }


def prep_ffn(p):
    g, bta = _np(p['ln']['g']), _np(p['ln']['b'])
    w1, b1 = _np(p['fc1']['w']), _np(p['fc1']['b'])
    w2, b2 = _np(p['fc2']['w']), _np(p['fc2']['b'])
    return {
        'w1': (g[:, None] * w1).astype(bf16),
        'b1': bta @ w1 + b1,
        'w2': w2.astype(bf16),
        'b2': b2,
    }


def patchify(img):
    """[C, 120, 120] -> [C*64, 225] fp32 (x0 transposed)."""
    C = img.shape[0]
    hh = RES // PATCH
    x = img.reshape(C, hh, PATCH, hh, PATCH).transpose(1, 3, 0, 2, 4)
    x = x.reshape(hh * hh, C * PATCH * PATCH)
    return np.ascontiguousarray(x.T.astype(np.float32))


def prep_alibi(attn_bias):
    """exp(bias) tiles, keys on partitions; padded keys get 0."""
    b = _np(attn_bias)[0]
    out = np.zeros((P, 2 * HEADS, NP), np.float32)
    for h in range(HEADS):
        bt = np.exp(b[h].T)
        out[:, 2 * h + 0, :] = bt[0:128, :]
        out[0:97, 2 * h + 1, :] = bt[128:225, :]
    return out.astype(bf16)


class BiasBlob:
    def __init__(self):
        self.cols = []
        self.offsets = {}

    def add(self, name, vec):
        vec = _np(vec)
        kt = vec.shape[0] // P
        self.offsets[name] = (len(self.cols), kt)
        arr = vec.reshape(kt, P).T
        for j in range(kt):
            self.cols.append(arr[:, j])

    def plan(self, name, n):
        kt = n // P
        self.offsets[name] = (len(self.cols), kt)
        for _ in range(kt):
            self.cols.append(None)

    def array(self):
        return np.stack(self.cols, axis=1).astype(np.float32)


def _blob_walk(blob, d1, d2, dc, add_fns=None):
    """Shared layout walk; add_fns = dict of callables to emit data."""
    A = add_fns

    def put(name, n, data_fn=None):
        if A is None:
            blob.plan(name, n)
        else:
            blob.add(name, data_fn())

    return put


def build_host_inputs(sar_imgs, opt_imgs, attn_bias, params, depths):
    d1, d2, dc = depths
    blob = BiasBlob()
    wblob = BiasBlob()
    consts = []
    shared = {}

    def add_w(name, arr):
        shared[name] = np.ascontiguousarray(arr)

    def add_bar(name, w, b):
        wblob.add(name, w.astype(np.float32).sum(axis=1))
        consts.append(float(b.sum()))

    def do_vit(tag, vp, depth):
        win = _np(vp['in']['w'])
        add_w(f'{tag}_inw', win.astype(bf16))
        blob.add(f'{tag}_inb', _np(vp['in']['b']))
        add_bar(f'{tag}_inwb', win, _np(vp['in']['b']))
        for i, L in enumerate(vp['layers'][:depth]):
            a = prep_attn(L['attn'])
            f = prep_ffn(L['ffn'])
            add_w(f'{tag}{i}a_wqk', a['wqk']); blob.add(f'{tag}{i}a_bqk', a['bqk'])
            add_w(f'{tag}{i}a_wv', a['wv'])
            add_w(f'{tag}{i}a_wo', a['wo']); blob.add(f'{tag}{i}a_bo', a['bo'])
            add_bar(f'{tag}{i}a_wob', a['wo'], a['bo'])
            add_w(f'{tag}{i}f_w1', f['w1']); blob.add(f'{tag}{i}f_b1', f['b1'])
            add_w(f'{tag}{i}f_w2', f['w2']); blob.add(f'{tag}{i}f_b2', f['b2'])
            add_bar(f'{tag}{i}f_w2b', f['w2'], f['b2'])
        blob.add(f'{tag}_lng', _np(vp['ln']['g']))
        blob.add(f'{tag}_lnb', _np(vp['ln']['b']))

    do_vit('s1', params['s1'], d1)
    do_vit('s2', params['s2'], d2)

    cp = params['cross']
    for i, L in enumerate(cp['layers'][:dc]):
        a = prep_attn(L['sa'])
        x = prep_xattn(L['ca'])
        f = prep_ffn(L['ffn'])
        add_w(f'c{i}s_wqk', a['wqk']); blob.add(f'c{i}s_bqk', a['bqk'])
        add_w(f'c{i}s_wv', a['wv'])
        add_w(f'c{i}s_wo', a['wo']); blob.add(f'c{i}s_bo', a['bo'])
        add_bar(f'c{i}s_wob', a['wo'], a['bo'])
        add_w(f'c{i}x_wq', x['wq']); blob.add(f'c{i}x_bq', x['bq'])
        add_w(f'c{i}x_wk', x['wk']); blob.add(f'c{i}x_bk', x['bk'])
        add_w(f'c{i}x_wv', x['wv'])
        add_w(f'c{i}x_wo', x['wo']); blob.add(f'c{i}x_bo', x['bo'])
        add_bar(f'c{i}x_wob', x['wo'], x['bo'])
        add_w(f'c{i}f_w1', f['w1']); blob.add(f'c{i}f_b1', f['b1'])
        add_w(f'c{i}f_w2', f['w2']); blob.add(f'c{i}f_b2', f['b2'])
        add_bar(f'c{i}f_w2b', f['w2'], f['b2'])
    blob.add('c_lng', _np(cp['ln']['g']))
    blob.add('c_lnb', _np(cp['ln']['b']))

    for tag, gp in (('g1', params['s1_gap']), ('g2', params['s2_gap'])):
        g = prep_ffn(gp)
        add_w(f'{tag}_w1', g['w1']); blob.add(f'{tag}_b1', g['b1'])
        add_w(f'{tag}_w2', g['w2']); blob.add(f'{tag}_b2', g['b2'])

    shared['biases'] = blob.array()
    shared['wbars'] = blob_to_bf16(wblob)
    shared['sumconsts'] = np.asarray(consts, np.float32)[None, :]
    shared['alibi'] = prep_alibi(attn_bias)

    sar = _np(sar_imgs)
    opt = _np(opt_imgs)
    per_core = [{'x0sar': patchify(sar[c]), 'x0opt': patchify(opt[c])}
                for c in range(N_CORES)]
    return shared, per_core, blob.offsets


def blob_to_bf16(blob):
    return blob.array().astype(bf16)


def plan_bias_offsets(depths):
    d1, d2, dc = depths
    blob = BiasBlob()
    wblob = BiasBlob()
    consts = []

    def plan_bar(name, n):
        wblob.plan(name, n)
        consts.append(name)

    def do_vit(tag, depth):
        blob.plan(f'{tag}_inb', DIM)
        plan_bar(f'{tag}_inwb', P if tag == 's1' else DIM)
        for i in range(depth):
            blob.plan(f'{tag}{i}a_bqk', 2 * HEADS * DHP)
            blob.plan(f'{tag}{i}a_bo', DIM)
            plan_bar(f'{tag}{i}a_wob', HEADS * DHP)
            blob.plan(f'{tag}{i}f_b1', FFN)
            blob.plan(f'{tag}{i}f_b2', DIM)
            plan_bar(f'{tag}{i}f_w2b', FFN)
        blob.plan(f'{tag}_lng', DIM)
        blob.plan(f'{tag}_lnb', DIM)

    do_vit('s1', d1)
    do_vit('s2', d2)
    for i in range(dc):
        blob.plan(f'c{i}s_bqk', 2 * HEADS * DHP)
        blob.plan(f'c{i}s_bo', DIM)
        plan_bar(f'c{i}s_wob', HEADS * DHP)
        blob.plan(f'c{i}x_bq', HEADS * DHP)
        blob.plan(f'c{i}x_bk', HEADS * DHP)
        blob.plan(f'c{i}x_bo', DIM)
        plan_bar(f'c{i}x_wob', HEADS * DHP)
        blob.plan(f'c{i}f_b1', FFN)
        blob.plan(f'c{i}f_b2', DIM)
        plan_bar(f'c{i}f_w2b', FFN)
    blob.plan('c_lng', DIM)
    blob.plan('c_lnb', DIM)
    for tag in ('g1', 'g2'):
        blob.plan(f'{tag}_b1', FFN)
        blob.plan(f'{tag}_b2', DIM)
    cidx = {n: i for i, n in enumerate(consts)}
    return blob.offsets, len(blob.cols), wblob.offsets, len(wblob.cols), cidx


# ---------------------------------------------------------------- device build
def build_program(depths):
    d1, d2, dc = depths
    bias_off, bias_cols, wbar_off, wbar_cols, cidx = plan_bias_offsets(depths)
    nc = bacc.Bacc("TRN2", target_bir_lowering=False, debug=False,
                   num_devices=N_CORES)

    din = {}

    def dram_in(name, shape, dtype):
        din[name] = nc.dram_tensor(name, list(shape), dtype, kind="ExternalInput")

    dram_in('x0sar', (P, NP), dt.float32)
    dram_in('x0opt', (DIM, NP), dt.float32)
    dram_in('biases', (P, bias_cols), dt.float32)
    dram_in('wbars', (P, wbar_cols), dt.bfloat16)
    dram_in('sumconsts', (1, len(cidx)), dt.float32)
    dram_in('alibi', (P, 2 * HEADS, NP), dt.bfloat16)

    def dram_w(name, shape):
        dram_in(name, shape, dt.bfloat16)

    def vit_weights(tag, depth):
        dram_w(f'{tag}_inw', (P if tag == 's1' else DIM, DIM))
        for i in range(depth):
            dram_w(f'{tag}{i}a_wqk', (DIM, 2 * HEADS * DHP))
            dram_w(f'{tag}{i}a_wv', (DIM, DIM))
            dram_w(f'{tag}{i}a_wo', (HEADS * DHP, DIM))
            dram_w(f'{tag}{i}f_w1', (DIM, FFN))
            dram_w(f'{tag}{i}f_w2', (FFN, DIM))

    vit_weights('s1', d1)
    vit_weights('s2', d2)
    for i in range(dc):
        dram_w(f'c{i}s_wqk', (DIM, 2 * HEADS * DHP))
        dram_w(f'c{i}s_wv', (DIM, DIM))
        dram_w(f'c{i}s_wo', (HEADS * DHP, DIM))
        dram_w(f'c{i}x_wq', (DIM, HEADS * DHP))
        dram_w(f'c{i}x_wk', (DIM, HEADS * DHP))
        dram_w(f'c{i}x_wv', (DIM, DIM))
        dram_w(f'c{i}x_wo', (HEADS * DHP, DIM))
        dram_w(f'c{i}f_w1', (DIM, FFN))
        dram_w(f'c{i}f_w2', (FFN, DIM))
    for tag in ('g1', 'g2'):
        dram_w(f'{tag}_w1', (DIM, FFN))
        dram_w(f'{tag}_w2', (FFN, DIM))

    douts = {}
    for name, shape in (('sar_enc', (NP, DIM)), ('opt_enc', (NP, DIM)),
                        ('joint', (NP, DIM)), ('sar_gap', (DIM,)),
                        ('opt_gap', (DIM,)), ('joint_gap', (DIM,))):
        douts[name] = nc.dram_tensor(name, list(shape), dt.float32,
                                     kind="ExternalOutput")

    from contextlib import ExitStack
    import itertools
    _ctr = itertools.count()

    MM_PHASES.clear()
    _orig_mm = nc.tensor.matmul

    def _mm(*a, **k):
        MM_PHASES.append(_CUR_PHASE[0])
        return _orig_mm(*a, **k)
    nc.tensor.matmul = _mm

    with TileContext(nc) as tc, ExitStack() as es:
        cst = es.enter_context(tc.tile_pool(name="cst", bufs=1))
        persist = es.enter_context(tc.tile_pool(name="persist", bufs=1))
        act = es.enter_context(tc.tile_pool(name="act", bufs=2))
        small = es.enter_context(tc.tile_pool(name="small", bufs=3))
        headp = es.enter_context(tc.tile_pool(name="headp", bufs=6))
        probp = es.enter_context(tc.tile_pool(name="probp", bufs=16))
        big = es.enter_context(tc.tile_pool(name="big", bufs=1))
        wp768 = es.enter_context(tc.tile_pool(name="wp768", bufs=30))
        wp1024 = es.enter_context(tc.tile_pool(name="wp1024", bufs=12))
        ps_mm = es.enter_context(tc.tile_pool(name="ps_mm", bufs=2, space="PSUM"))
        ps_sc = es.enter_context(tc.tile_pool(name="ps_sc", bufs=2, space="PSUM"))
        ps_av = es.enter_context(tc.tile_pool(name="ps_av", bufs=2, space="PSUM"))
        ps_bc = es.enter_context(tc.tile_pool(name="ps_bc", bufs=2, space="PSUM"))
        ps_pool = {"mm": ps_mm, "sc": ps_sc, "av": ps_av}

        # ---- constants
        bias_sb = cst.tile([P, bias_cols], dt.float32)
        nc.sync.dma_start(bias_sb[:], din['biases'][:])
        wbar_sb = cst.tile([P, wbar_cols], dt.bfloat16)
        nc.sync.dma_start(wbar_sb[:], din['wbars'][:])
        sconst_sb = cst.tile([1, len(cidx)], dt.float32)
        nc.sync.dma_start(sconst_sb[:], din['sumconsts'][:])
        alibi_sb = cst.tile([P, 2 * HEADS, NP], dt.bfloat16)
        nc.sync.dma_start(alibi_sb[:], din['alibi'][:])
        ones_col32 = cst.tile([P, 1], dt.float32)
        nc.vector.memset(ones_col32[:], 1.0)
        ones_col16 = cst.tile([P, 1], dt.bfloat16)
        nc.vector.memset(ones_col16[:], 1.0)
        ones_row32 = cst.tile([1, P], dt.float32)
        nc.vector.memset(ones_row32[:], 1.0)
        ones_row16 = cst.tile([1, P], dt.bfloat16)
        nc.vector.memset(ones_row16[:], 1.0)
        ident = cst.tile([P, P], dt.float32)
        make_identity(nc, ident[:])
        eps_sb = cst.tile([1, 1], dt.float32)
        nc.vector.memset(eps_sb[:], EPS)

        VS = DHP + 1  # 65: V cols 0..47, zeros 48..63, ones col at 64
        v_sb = persist.tile([P, 2, HEADS * VS], dt.bfloat16)
        nc.vector.memset(v_sb[:], 0.0)
        nc.vector.memset(
            v_sb[:].rearrange("p t (h c) -> p t h c", c=VS)[:, :, :, DHP], 1.0)
        outT_sb = persist.tile([P, HEADS // 2, NP], dt.bfloat16)
        nc.vector.memset(outT_sb[:], 0.0)

        def bias_ap(name, j):
            off, kt = bias_off[name]
            assert j < kt, (name, j, kt)
            return bias_sb[:, off + j, None]

        def update_sums(sum_tile, bar_name, rhs, kt_in):
            """sum_tile = colsum(proj) + const + sum_tile, via wbar matmul."""
            off, kt = wbar_off[bar_name]
            assert kt == kt_in, (bar_name, kt, kt_in)
            psig = ps_bc.tile([P, 512], dt.float32, tag="bc",
                              name=f"psig_{next(_ctr)}")[0:1, :NP]
            for k in range(kt_in):
                nc.tensor.matmul(psig[:], wbar_sb[:, off + k, None],
                                 rhs[:, k, :], start=(k == 0),
                                 stop=(k == kt_in - 1))
            ci = cidx[bar_name]
            nc.vector.scalar_tensor_tensor(sum_tile[:], psig[:],
                                           sconst_sb[0:1, ci:ci + 1],
                                           sum_tile[:], ALU.add, ALU.add)

        def init_sums(sum_tile, bar_name, rhs, kt_in):
            off, kt = wbar_off[bar_name]
            psig = ps_bc.tile([P, 512], dt.float32, tag="bc",
                              name=f"psii_{next(_ctr)}")[0:1, :NP]
            for k in range(kt_in):
                nc.tensor.matmul(psig[:], wbar_sb[:, off + k, None],
                                 rhs[:, k, :], start=(k == 0),
                                 stop=(k == kt_in - 1))
            ci = cidx[bar_name]
            nc.vector.tensor_scalar(sum_tile[:], psig[:],
                                    1.0, sconst_sb[0:1, ci:ci + 1],
                                    ALU.mult, ALU.add)

        def fresh_sums(xT, sum_tile):
            """sum_tile = colsum(xT) (fp32 matmuls)."""
            psig = ps_bc.tile([P, 512], dt.float32, tag="bc",
                              name=f"psif_{next(_ctr)}")[0:1, :NP]
            for k in range(KT):
                nc.tensor.matmul(psig[:], ones_col32[:], xT[:, k, :],
                                 start=(k == 0), stop=(k == KT - 1))
            nc.vector.tensor_copy(sum_tile[:], psig[:])

        # ---------------------------------------------------------- helpers
        def w_group(name, kt_in, c0, c1):
            """DMA chunks [P, c1-c0] for all kt of weight columns [c0:c1)."""
            ap = din[name].rearrange("(ko p) m -> p ko m", p=P)
            width = c1 - c0
            tag = "w768" if width <= DIM else "w1024"
            pool = wp768 if width <= DIM else wp1024
            chunks = []
            for k in range(kt_in):
                c = pool.tile([P, DIM if width <= DIM else 1024],
                              dt.bfloat16, tag=tag,
                              name=f"wchunk_{next(_ctr)}")[:, :width]
                nc.sync.dma_start(c[:], ap[:, k, c0:c1])
                chunks.append(c)
            return chunks

        def linear(xn, name, kt_in, m_out, consumer, group_cols=None):
            """psum_m = sum_k W[:, m-block].T @ xn[:, k, :]; consumer(m, psum).
            Streams weight column-groups of group_cols (default min(m_out,1024))."""
            gc = group_cols or min(m_out, 1024)
            for c0 in range(0, m_out, gc):
                chunks = w_group(name, kt_in, c0, min(c0 + gc, m_out))
                for mi in range((min(c0 + gc, m_out) - c0) // P):
                    m = c0 // P + mi
                    pt = ps_mm.tile([P, 512], dt.float32, tag="mm",
                                    name=f"ptmm_{next(_ctr)}")[:, :NP]
                    for k in range(kt_in):
                        nc.tensor.matmul(pt[:], chunks[k][:, mi * P:(mi + 1) * P],
                                         xn[:, k, :], start=(k == 0),
                                         stop=(k == kt_in - 1))
                    consumer(m, pt)

        def linear_ktg(xn, name, kt_in, m_out, consumer, kgroup=KT,
                       nfree=NP):
            """kt-grouped linear for tall weights (fc2): m_out//P open psums,
            weights streamed kgroup k-chunks at a time."""
            nm = m_out // P
            psums = []
            for m in range(nm):
                pool = (ps_mm, ps_sc, ps_av)[m % 3]
                psums.append(pool.tile([P, 512], dt.float32,
                                       tag=("mm", "sc", "av")[m % 3],
                                       name=f"ptk_{next(_ctr)}")[:, :nfree])
            ap = din[name].rearrange("(ko p) m -> p ko m", p=P)
            for g0 in range(0, kt_in, kgroup):
                for k in range(g0, min(g0 + kgroup, kt_in)):
                    c = wp768.tile([P, DIM], dt.bfloat16, tag="w768")
                    nc.sync.dma_start(c[:], ap[:, k, :])
                    for m in range(nm):
                        nc.tensor.matmul(psums[m][:], c[:, m * P:(m + 1) * P],
                                         xn[:, k, :], start=(k == 0),
                                         stop=(k == kt_in - 1))
            for m in range(nm):
                consumer(m, psums[m])

        def ln_raw(xT, sum_ap, out_dtype=dt.bfloat16, gamma=None, beta=None,
                   out_pool=None, out_tag="xn"):
            set_phase("ln")
            pool = out_pool or act
            xsq = act.tile([P, KT, NP], dt.bfloat16, tag="xsq")
            for k in range(KT):
                nc.scalar.square(xsq[:, k, :], xT[:, k, :])
            pstat = ps_bc.tile([1, 512], dt.float32, tag="bc")
            for k in range(KT):
                nc.tensor.matmul(pstat[0:1, NP:2 * NP], ones_col16[:],
                                 xsq[:, k, :], start=(k == 0),
                                 stop=(k == KT - 1))
            st = small.tile([1, 4 * NP], dt.float32, tag="stat")
            mu = st[0:1, 0:NP]
            e2 = st[0:1, NP:2 * NP]   # scaled in-place; becomes var
            r = st[0:1, NP:2 * NP]    # rstd writes over var slot
            m2 = st[0:1, 3 * NP:4 * NP]
            nc.vector.tensor_scalar_mul(mu, sum_ap[:], 1.0 / DIM)
            nc.vector.tensor_mul(m2, mu, mu)
            nc.vector.tensor_scalar_mul(e2, pstat[0:1, NP:2 * NP], 1.0 / DIM)
            nc.vector.tensor_sub(e2, e2, m2)
            nc.scalar.activation(m2, e2, AF.Ln, bias=eps_sb[0:1, 0:1])
            nc.scalar.activation(r, m2, AF.Exp, bias=0.0, scale=-0.5)
            pb = ps_bc.tile([P, 512], dt.float32, tag="bc")
            nc.tensor.matmul(pb[:, 0:2 * NP], ones_row32[0:1, :],
                             st[0:1, 0:2 * NP], start=True, stop=True)
            xn = pool.tile([P, KT, NP], out_dtype, tag=out_tag)
            for k in range(KT):
                nc.vector.tensor_sub(xn[:, k, :], xT[:, k, :], pb[:, 0:NP])
                nc.vector.tensor_mul(xn[:, k, :], xn[:, k, :], pb[:, NP:2 * NP])
            if gamma is not None:
                for k in range(KT):
                    nc.vector.tensor_scalar(xn[:, k, :], xn[:, k, :],
                                            bias_ap(gamma, k),
                                            bias_ap(beta, k),
                                            ALU.mult, ALU.add)
            return xn

        def attention(xT, xn, pfx, sums, kv_xn=None, cross=False):
            kv_src = kv_xn if kv_xn is not None else xn
            set_phase("attn_qk")
            qkT = big.tile([P, 16, NPAD], dt.bfloat16, tag="qkT")
            nc.vector.memset(qkT[:, :, NP:], 0.0)

            if not cross:
                def qk_consume(m, pt):
                    nc.scalar.activation(qkT[:, m, 0:NP], pt[:], AF.Identity,
                                         bias=bias_ap(f'{pfx}_bqk', m), scale=1.0)
                linear(xn, f'{pfx}_wqk', KT, 2 * HEADS * DHP, qk_consume)
                wo_name, bo_name, wv_name = f'{pfx}_wo', f'{pfx}_bo', f'{pfx}_wv'
            else:
                def q_consume(m, pt):
                    nc.scalar.activation(qkT[:, m, 0:NP], pt[:], AF.Identity,
                                         bias=bias_ap(f'{pfx}_bq', m), scale=1.0)
                linear(xn, f'{pfx}_wq', KT, HEADS * DHP, q_consume)

                def k_consume(m, pt):
                    nc.scalar.activation(qkT[:, 8 + m, 0:NP], pt[:], AF.Identity,
                                         bias=bias_ap(f'{pfx}_bk', m), scale=1.0)
                linear(kv_src, f'{pfx}_wk', KT, HEADS * DHP, k_consume)
                wo_name, bo_name, wv_name = f'{pfx}_wo', f'{pfx}_bo', f'{pfx}_wv'

            # V natural [keys, 768]
            set_phase("attn_v")
            wv_chunks = w_group(wv_name, KT, 0, DIM)
            for mt, rows in ((0, P), (1, NP - P)):
                for nt in range(2):
                    pv = ps_mm.tile([P, 512], dt.float32, tag="mm",
                                    name=f"ptv_{next(_ctr)}")[:, :384]
                    for k in range(KT):
                        nc.tensor.matmul(
                            pv[:rows, :],
                            kv_src[:, k, mt * P: mt * P + rows],
                            wv_chunks[k][:, nt * 384:(nt + 1) * 384],
                            start=(k == 0), stop=(k == KT - 1))
                    for j in range(8):
                        h = nt * 8 + j
                        eng = nc.vector if j % 2 == 0 else nc.scalar
                        if j % 2 == 0:
                            nc.vector.tensor_copy(
                                v_sb[:rows, mt, h * VS: h * VS + DH],
                                pv[:rows, j * DH:(j + 1) * DH])
                        else:
                            nc.scalar.copy(
                                v_sb[:rows, mt, h * VS: h * VS + DH],
                                pv[:rows, j * DH:(j + 1) * DH])

            set_phase("attn_heads")
            for h in range(HEADS):
                rb = (h % 2) * DHP
                qm, km = h // 2, 8 + h // 2
                if h % 2 == 0:
                    po = ps_av.tile([P, NP], dt.float32, tag="av",
                                    name=f"po_{next(_ctr)}")
                else:
                    po = ps_mm.tile([P, 512], dt.float32, tag="mm",
                                    name=f"po_{next(_ctr)}")[:, :NP]
                expts = []
                for kt2 in range(2):
                    pscr = ps_sc.tile([P, NP], dt.float32, tag="sc")
                    nc.tensor.matmul(pscr[:],
                                     qkT[rb:rb + DHP, km, kt2 * P:(kt2 + 1) * P],
                                     qkT[rb:rb + DHP, qm, 0:NP],
                                     start=True, stop=True)
                    eT = headp.tile([P, NP], dt.bfloat16, tag="eT")
                    nc.scalar.activation(eT[:], pscr[:], AF.Exp)
                    eT2 = probp.tile([P, NP], dt.bfloat16, tag="eT2")
                    eng = nc.vector if kt2 == 0 else nc.gpsimd
                    eng.tensor_mul(eT2[:], eT[:],
                                   alibi_sb[:, 2 * h + kt2, :])
                    expts.append(eT2)
                for kt2 in range(2):
                    nc.tensor.matmul(
                        po[0:VS, :],
                        v_sb[:, kt2, h * VS:(h + 1) * VS],
                        expts[kt2][:], start=(kt2 == 0), stop=(kt2 == 1))
                rec = headp.tile([1, NP], dt.float32, tag="rec")
                nc.vector.reciprocal(rec[:], po[DHP:DHP + 1, :])
                pbc = ps_bc.tile([P, 512], dt.float32, tag="bc")
                nc.tensor.matmul(pbc[0:DH, 0:NP], ones_row32[0:1, 0:DH],
                                 rec[0:1, :], start=True, stop=True)
                recb = headp.tile([DH, NP], dt.float32, tag="recb")
                nc.scalar.copy(recb[:], pbc[0:DH, 0:NP])
                nc.vector.tensor_mul(outT_sb[rb:rb + DH, h // 2, :],
                                     po[0:DH, :], recb[:])

            set_phase("attn_out")
            xT_new = act.tile([P, KT, NP], dt.float32, tag="xT")

            def out_consume(m, pt):
                nc.vector.scalar_tensor_tensor(xT_new[:, m, :], pt[:],
                                               bias_ap(bo_name, m),
                                               xT[:, m, :], ALU.add, ALU.add)
            linear(outT_sb, wo_name, HEADS * DHP // P, DIM, out_consume,
                   group_cols=DIM)
            update_sums(sums, f'{pfx}_wob', outT_sb, HEADS * DHP // P)
            return xT_new

        def ffn(xT, pfx, sums):
            xn = ln_raw(xT, sums)
            set_phase("ffn_fc1")
            hT = big.tile([P, FKT, NP], dt.bfloat16, tag="hT")

            def fc1_consume(m, pt):
                nc.scalar.activation(hT[:, m, :], pt[:], AF.Gelu,
                                     bias=bias_ap(f'{pfx}_b1', m), scale=1.0)
            linear(xn, f'{pfx}_w1', KT, FFN, fc1_consume, group_cols=DIM)

            set_phase("ffn_fc2")
            xT_new = act.tile([P, KT, NP], dt.float32, tag="xT")

            def fc2_consume(m, pt):
                nc.vector.scalar_tensor_tensor(xT_new[:, m, :], pt[:],
                                               bias_ap(f'{pfx}_b2', m),
                                               xT[:, m, :], ALU.add, ALU.add)
            linear(hT, f'{pfx}_w2', FKT, DIM, fc2_consume, group_cols=DIM)
            update_sums(sums, f'{pfx}_w2b', hT, FKT)
            return xT_new

        def emit_enc_outputs(encT, out_name):
            set_phase("outputs")
            for m in range(KT):
                for t2, rows in ((0, P), (1, NP - P)):
                    ptr = ps_bc.tile([P, 512], dt.float32, tag="bc")
                    nc.tensor.transpose(ptr[:rows, 0:P],
                                        encT[:, m, t2 * P: t2 * P + rows],
                                        ident[:])
                    nat = small.tile([P, P], dt.float32, tag="nat")
                    nc.vector.tensor_copy(nat[:rows, :], ptr[:rows, 0:P])
                    nc.sync.dma_start(
                        douts[out_name][t2 * P: t2 * P + rows,
                                        m * P:(m + 1) * P],
                        nat[:rows, :])

        def gap_head(encT, tag, out_name):
            set_phase("gap")
            mt = small.tile([P, KT, 1], dt.float32, tag="gapm")
            nc.vector.reduce_sum(out=mt[:, :, 0], in_=encT[:], axis=AX.X)
            nc.vector.tensor_scalar_mul(mt[:], mt[:], 1.0 / NP)
            msq = small.tile([P, KT, 1], dt.bfloat16, tag="gapsq")
            nc.scalar.square(msq[:], mt[:])
            pstat = ps_bc.tile([1, 512], dt.float32, tag="bc")
            for k in range(KT):
                nc.tensor.matmul(pstat[0:1, 0:1], ones_col32[:], mt[:, k, :],
                                 start=(k == 0), stop=(k == KT - 1))
            for k in range(KT):
                nc.tensor.matmul(pstat[0:1, 1:2], ones_col16[:], msq[:, k, :],
                                 start=(k == 0), stop=(k == KT - 1))
            st = small.tile([1, 4], dt.float32, tag="gapst")
            mu, r, tmp = st[0:1, 0:1], st[0:1, 1:2], st[0:1, 2:3]
            nc.vector.tensor_scalar_mul(mu, pstat[0:1, 0:1], 1.0 / DIM)
            nc.vector.tensor_scalar_mul(tmp, pstat[0:1, 1:2], 1.0 / DIM)
            m2 = st[0:1, 3:4]
            nc.vector.tensor_mul(m2, mu, mu)
            nc.vector.tensor_sub(tmp, tmp, m2)
            nc.vector.tensor_scalar_add(tmp, tmp, EPS)
            nc.scalar.activation(m2, tmp, AF.Ln)
            nc.scalar.activation(r, m2, AF.Exp, bias=0.0, scale=-0.5)
            pb = ps_bc.tile([P, 512], dt.float32, tag="bc")
            nc.tensor.matmul(pb[:, 0:2], ones_row32[0:1, :], st[0:1, 0:2],
                             start=True, stop=True)
            xn = small.tile([P, KT, 1], dt.bfloat16, tag="gapxn")
            for k in range(KT):
                nc.vector.tensor_sub(xn[:, k, :], mt[:, k, :], pb[:, 0:1])
                nc.vector.tensor_mul(xn[:, k, :], xn[:, k, :], pb[:, 1:2])
            hT = small.tile([P, FKT, 1], dt.bfloat16, tag="gaph")

            def g1_consume(m, pt):
                nc.scalar.activation(hT[:, m, 0:1], pt[:, 0:1], AF.Gelu,
                                     bias=bias_ap(f'{tag}_b1', m), scale=1.0)

            def lin1(m_lo, m_hi, chunks):
                for m in range(m_lo, m_hi):
                    pt = ps_mm.tile([P, 512], dt.float32, tag="mm",
                                    name=f"ptg_{next(_ctr)}")[:, 0:1]
                    for k in range(KT):
                        nc.tensor.matmul(pt[:], chunks[k][:, (m - m_lo) * P:
                                                          (m - m_lo + 1) * P],
                                         xn[:, k, :], start=(k == 0),
                                         stop=(k == KT - 1))
                    g1_consume(m, pt)

            for g in range(4):
                chunks = w_group(f'{tag}_w1', KT, g * DIM, (g + 1) * DIM)
                lin1(g * KT, (g + 1) * KT, chunks)

            gout = small.tile([P, KT, 1], dt.float32, tag="gapo")

            def g2_consume(m, pt):
                nc.scalar.activation(gout[:, m, 0:1], pt[:, 0:1], AF.Identity,
                                     bias=bias_ap(f'{tag}_b2', m), scale=1.0)
            linear_ktg(hT, f'{tag}_w2', FKT, DIM, g2_consume, nfree=1)
            nc.sync.dma_start(douts[out_name].rearrange("(ko p) -> p ko", p=P),
                              gout[:, :, 0])

        # ---------------------------------------------------------- encoders
        def run_vit(tag, depth, x0_name, in_kt):
            x0 = persist.tile([P, KT, NP], dt.float32, tag="x0",
                              name=f"x0_{tag}")[:, :in_kt, :]
            nc.sync.dma_start(
                x0[:], din[x0_name].rearrange("(ko p) n -> p ko n", p=P))
            x0b = persist.tile([P, KT, NP], dt.bfloat16, tag="x0b",
                               name=f"x0b_{tag}")[:, :in_kt, :]
            nc.vector.tensor_copy(x0b[:], x0[:])
            xT = act.tile([P, KT, NP], dt.float32, tag="xT")

            def in_consume(m, pt):
                nc.scalar.activation(xT[:, m, :], pt[:], AF.Identity,
                                     bias=bias_ap(f'{tag}_inb', m), scale=1.0)
            linear(x0b, f'{tag}_inw', in_kt, DIM, in_consume, group_cols=DIM)
            sums = persist.tile([1, NP], dt.float32, tag=f"sums_{tag}",
                                name=f"sums_{tag}")
            init_sums(sums, f'{tag}_inwb', x0b, in_kt)

            for i in range(depth):
                xn = ln_raw(xT, sums)
                xT = attention(xT, xn, f'{tag}{i}a', sums)
                xT = ffn(xT, f'{tag}{i}f', sums)
            encT = ln_raw(xT, sums, out_dtype=dt.float32, gamma=f'{tag}_lng',
                          beta=f'{tag}_lnb', out_pool=persist,
                          out_tag=f'enc_{tag}')
            return encT

        sar_encT = run_vit('s1', d1, 'x0sar', 1)
        emit_enc_outputs(sar_encT, 'sar_enc')
        gap_head(sar_encT, 'g1', 'sar_gap')

        opt_encT = run_vit('s2', d2, 'x0opt', 6)
        emit_enc_outputs(opt_encT, 'opt_enc')
        gap_head(opt_encT, 'g2', 'opt_gap')

        sums_cn = persist.tile([1, NP], dt.float32, tag="sums_cn")
        fresh_sums(opt_encT, sums_cn)
        cn_raw = ln_raw(opt_encT, sums_cn, out_pool=persist, out_tag="cn_raw")

        sums_j = persist.tile([1, NP], dt.float32, tag="sums_j")
        fresh_sums(sar_encT, sums_j)
        xT = sar_encT
        for i in range(dc):
            xn = ln_raw(xT, sums_j)
            xT = attention(xT, xn, f'c{i}s', sums_j)
            xn = ln_raw(xT, sums_j)
            xT = attention(xT, xn, f'c{i}x', sums_j, kv_xn=cn_raw, cross=True)
            xT = ffn(xT, f'c{i}f', sums_j)
        jointT = ln_raw(xT, sums_j, out_dtype=dt.float32, gamma='c_lng',
                        beta='c_lnb', out_pool=persist, out_tag='enc_joint')
        emit_enc_outputs(jointT, 'joint')

        jm = small.tile([P, KT, 1], dt.float32, tag="gapm")
        nc.vector.reduce_sum(out=jm[:, :, 0], in_=jointT[:], axis=AX.X)
        nc.vector.tensor_scalar_mul(jm[:], jm[:], 1.0 / NP)
        nc.sync.dma_start(douts['joint_gap'].rearrange("(ko p) -> p ko", p=P),
                          jm[:, :, 0])

    nc.finalize()
    return nc


# ---------------------------------------------------------------- entrypoint
_CACHED = {}


def kernel(sar_imgs, opt_imgs, attn_bias, params):
    depths = _depths()
    shared, per_core, _ = build_host_inputs(sar_imgs, opt_imgs, attn_bias,
                                            params, depths)
    if depths not in _CACHED:
        _CACHED[depths] = build_program(depths)
    nc = _CACHED[depths]
    in_maps = [{**shared, **pc} for pc in per_core]
    res = run_bass_kernel_spmd(nc, in_maps, list(range(N_CORES)))
    outs = []
    for name in ('sar_enc', 'sar_gap', 'opt_enc', 'opt_gap', 'joint',
                 'joint_gap'):
        outs.append(np.stack([res.results[c][name] for c in range(N_CORES)]))
    return tuple(outs)
